# revision 11
# baseline (speedup 1.0000x reference)
"""CBAM kernel for Trainium2, 8-way batch-parallel SPMD.

Computes out = x^2 * (att_c[b,c] + sigmoid(conv(spatial_stats))[b,l]) where
att_c = sigmoid(mlp(mean_L x) + mlp(max_L x)), matching the CBAM reference.

Layout per core: 4 batches; each batch x[4096, 256] lives in SBUF as one
[128, 8192] tensor (partition = l % 128, free column = 256*(l//128) + c).
Engine split per batch:
  PE   : channel-sum (ones-matmul accumulation), transposes, MLP, conv
         (conv over L is a banded-Toeplitz matmul with host-built weights)
  ACT  : spatial sums (copy w/ accum_out), squares, sigmoids, relu
  DVE  : spatial max (one 3D reduce), max-tree folds, final fused
         (att + sig) * x^2 via scalar_tensor_tensor
  POOL : first max-tree fold
"""

import numpy as np
from contextlib import ExitStack

import concourse.bacc as bacc
import concourse.bass as bass
import concourse.tile as tile
import concourse.mybir as mybir
from concourse.bass_utils import run_bass_kernel_spmd

AF = mybir.ActivationFunctionType
ALU = mybir.AluOpType
AX = mybir.AxisListType
FP32 = mybir.dt.float32

N_CORES = 8
B_FULL = 32
NB = B_FULL // N_CORES  # batches per core = 4
L = 4096
C = 256
HID = 16
P = 128
NT = L // P  # 32 L-tiles per batch
SQW = 2048   # ACT square slice width (8 tiles)

_CACHE: dict = {}


def _build_body(ctx: ExitStack, tc, out_d, x_d, w1_d, b1_d, w2b_d, cm_d, cc_d,
                ones_d, id_d):
    nc = tc.nc

    const = ctx.enter_context(tc.tile_pool(name="const", bufs=1))
    xpool = ctx.enter_context(tc.tile_pool(name="x", bufs=2))
    mpool = ctx.enter_context(tc.tile_pool(name="maxtree", bufs=2))
    spool = ctx.enter_context(tc.tile_pool(name="stats", bufs=2))
    sqpool = ctx.enter_context(tc.tile_pool(name="sq", bufs=3))
    opool = ctx.enter_context(tc.tile_pool(name="outt", bufs=8))
    dpool = ctx.enter_context(tc.tile_pool(name="dummy", bufs=2))
    apool = ctx.enter_context(tc.tile_pool(name="att", bufs=2))
    pacc = ctx.enter_context(tc.tile_pool(name="pacc", bufs=2, space="PSUM"))
    pwork = ctx.enter_context(tc.tile_pool(name="pwork", bufs=4, space="PSUM"))

    w1 = const.tile([P, 2 * (HID + 1)], FP32)
    nc.sync.dma_start(w1[:], w1_d[:])
    b1 = const.tile([HID + 1, 1], FP32)
    nc.sync.dma_start(b1[:], b1_d[:])
    w2b = const.tile([HID + 1, C], FP32)
    nc.sync.dma_start(w2b[:], w2b_d[:])
    cmain = const.tile([P, 2 * P], FP32)
    nc.sync.dma_start(cmain[:], cm_d[:])
    ccorn = const.tile([P, 4 * P], FP32)
    nc.sync.dma_start(ccorn[:], cc_d[:])
    ones = const.tile([P, P], FP32)
    nc.sync.dma_start(ones[:], ones_d[:])
    ident = const.tile([P, P], FP32)
    nc.sync.dma_start(ident[:], id_d[:])

    for b in range(NB):
        xb = xpool.tile([P, NT * C], FP32, tag="xb")
        for t in range(NT):
            nc.sync.dma_start(xb[:, C * t:C * (t + 1)],
                              x_d[b, P * t:P * (t + 1), :])

        # ---- channel sum over L: PSUM-accumulated ones-matmuls (PE) ----
        pcs = pacc.tile([1, C], FP32, tag="pcs")
        for t in range(NT):
            nc.tensor.matmul(pcs[:], ones[:, 0:1], xb[:, C * t:C * (t + 1)],
                             start=(t == 0), stop=(t == NT - 1),
                             skip_group_check=True)

        # ---- spatial sum over C: ACT copies with accum_out ----
        sum_s = spool.tile([P, NT], FP32, tag="sum_s")
        for t in range(NT):
            dummy = dpool.tile([P, C], FP32, tag="dummy")
            nc.scalar.activation(dummy[:], xb[:, C * t:C * (t + 1)],
                                 AF.Identity, accum_out=sum_s[:, t:t + 1])

        # ---- spatial max over C: one 3D reduce (DVE) ----
        max_s = spool.tile([P, NT], FP32, tag="max_s")
        nc.vector.tensor_reduce(max_s[:],
                                xb[:].rearrange("p (t c) -> p t c", c=C),
                                axis=AX.X, op=ALU.max)

        # ---- channel max over L: fold tree then transpose+reduce ----
        mb = mpool.tile([P, NT * C // 2], FP32, tag="mb")
        half = NT * C // 2  # 4096
        nc.vector.tensor_max(mb[:], xb[:, 0:half], xb[:, half:2 * half])
        w = half // 2
        while w >= C:
            nc.vector.tensor_max(mb[:, 0:w], mb[:, 0:w], mb[:, w:2 * w])
            w //= 2

        stats_cm = spool.tile([P, 4], FP32, tag="stats_cm")
        for h in range(2):
            pt = pwork.tile([P, P], FP32, tag="pwork")
            nc.tensor.transpose(pt[:], mb[:, P * h:P * (h + 1)], ident[:])
            nc.vector.tensor_reduce(stats_cm[:, 2 * h + 1:2 * h + 2], pt[:],
                                    axis=AX.X, op=ALU.max)

        avg_row = spool.tile([1, C], FP32, tag="avg_row")
        nc.scalar.activation(avg_row[:], pcs[:], AF.Copy, scale=1.0 / L)
        for h in range(2):
            pa = pwork.tile([P, 1], FP32, tag="pwork")
            nc.tensor.transpose(pa[:], avg_row[:, P * h:P * (h + 1)],
                                ident[0:1, 0:1])
            nc.scalar.activation(stats_cm[:, 2 * h:2 * h + 1], pa[:], AF.Copy)

        # ---- shared MLP: att logits broadcast over partitions via matmul ----
        # Row HID (=16) carries a constant: lhsT col 16 is zero, relu bias row
        # 16 is 1.0, so hsb[16, :] = 1, h2[16] = 2 — which multiplies the b2
        # row of w2b to add the 2*b2 term.
        ph = pwork.tile([HID + 1, 2], FP32, tag="pwork")
        nc.tensor.matmul(ph[:], w1[:, 0:HID + 1], stats_cm[:, 0:2],
                         start=True, stop=False, skip_group_check=True)
        nc.tensor.matmul(ph[:], w1[:, HID + 1:2 * (HID + 1)], stats_cm[:, 2:4],
                         start=False, stop=True, skip_group_check=True)
        hsb = spool.tile([HID + 1, 2], FP32, tag="hsb")
        nc.scalar.activation(hsb[:], ph[:], AF.Relu, bias=b1[:])
        h2 = spool.tile([HID + 1, 1], FP32, tag="h2")
        nc.vector.tensor_add(h2[:], hsb[:, 0:1], hsb[:, 1:2])
        h2r = spool.tile([HID + 1, P], FP32, tag="h2r")
        nc.vector.tensor_scalar_mul(h2r[:], ones[0:HID + 1, :], h2[:])
        po = pwork.tile([P, C], FP32, tag="pwork")
        nc.tensor.matmul(po[:], h2r[:], w2b[:], start=True, stop=True,
                         skip_group_check=True)
        att = apool.tile([P, C], FP32, tag="att")
        nc.scalar.activation(att[:], po[:], AF.Sigmoid)

        # ---- spatial conv over L: banded-Toeplitz matmuls ----
        pc = pwork.tile([P, NT], FP32, tag="pwork")
        nc.tensor.matmul(pc[:, :], cmain[:, 0:P], sum_s[:],
                         start=True, stop=False, skip_group_check=True)
        nc.tensor.matmul(pc[:, :], cmain[:, P:2 * P], max_s[:],
                         start=False, stop=False, skip_group_check=True)
        nc.tensor.matmul(pc[:, 1:NT], ccorn[:, 0:P], sum_s[:, 0:NT - 1],
                         start=False, stop=False, skip_group_check=True)
        nc.tensor.matmul(pc[:, 1:NT], ccorn[:, P:2 * P],
                         max_s[:, 0:NT - 1],
                         start=False, stop=False, skip_group_check=True)
        nc.tensor.matmul(pc[:, 0:NT - 1], ccorn[0:3, 2 * P:3 * P],
                         sum_s[0:3, 1:NT],
                         start=False, stop=False, skip_group_check=True)
        nc.tensor.matmul(pc[:, 0:NT - 1], ccorn[0:3, 3 * P:4 * P],
                         max_s[0:3, 1:NT],
                         start=False, stop=True, skip_group_check=True)
        sig = spool.tile([P, NT], FP32, tag="sig")
        nc.scalar.activation(sig[:], pc[:], AF.Sigmoid)

        # ---- final: out = (att + sig) * x^2 ----
        sqs = []
        for s in range(NT * C // SQW):
            sq = sqpool.tile([P, SQW], FP32, tag="sq")
            nc.scalar.activation(sq[:], xb[:, SQW * s:SQW * (s + 1)], AF.Square)
            sqs.append(sq)
        tps = SQW // C  # tiles per square slice
        for t in range(NT):
            ot = opool.tile([P, C], FP32, tag="ot")
            sq = sqs[t // tps]
            off = C * (t % tps)
            nc.vector.scalar_tensor_tensor(ot[:], att[:], sig[:, t:t + 1],
                                           sq[:, off:off + C],
                                           op0=ALU.add, op1=ALU.mult)
            nc.sync.dma_start(out_d[b, P * t:P * (t + 1), :], ot[:])


def _build_nc():
    nc = bacc.Bacc("TRN2", target_bir_lowering=False, debug=False,
                   enable_asserts=False, num_devices=N_CORES)
    x_d = nc.dram_tensor("xb", [NB, L, C], FP32, kind="ExternalInput").ap()
    w1_d = nc.dram_tensor("w1sb", [P, 2 * (HID + 1)], FP32, kind="ExternalInput").ap()
    b1_d = nc.dram_tensor("b1col", [HID + 1, 1], FP32, kind="ExternalInput").ap()
    w2b_d = nc.dram_tensor("w2b", [HID + 1, C], FP32, kind="ExternalInput").ap()
    cm_d = nc.dram_tensor("convmain", [P, 2 * P], FP32, kind="ExternalInput").ap()
    cc_d = nc.dram_tensor("convcorner", [P, 4 * P], FP32, kind="ExternalInput").ap()
    ones_d = nc.dram_tensor("ones", [P, P], FP32, kind="ExternalInput").ap()
    id_d = nc.dram_tensor("ident", [P, P], FP32, kind="ExternalInput").ap()
    out_d = nc.dram_tensor("out", [NB, L, C], FP32, kind="ExternalOutput").ap()

    with tile.TileContext(nc) as tc:
        with ExitStack() as ctx:
            _build_body(ctx, tc, out_d, x_d, w1_d, b1_d, w2b_d, cm_d, cc_d,
                        ones_d, id_d)
    nc.compile()
    return nc


def get_nc():
    if "nc" not in _CACHE:
        _CACHE["nc"] = _build_nc()
    return _CACHE["nc"]


def _prep_inputs(W1, b1, W2, b2, conv_w):
    """Host-side parameter preprocessing (shared across cores)."""
    W1 = np.asarray(W1, np.float32)
    W2 = np.asarray(W2, np.float32)
    b1 = np.asarray(b1, np.float32)
    b2 = np.asarray(b2, np.float32)
    conv_w = np.asarray(conv_w, np.float32)

    HB = HID + 1
    w1sb = np.zeros((P, 2 * HB), np.float32)
    for h in range(2):
        w1sb[:, HB * h:HB * h + HID] = W1[P * h:P * (h + 1), :]
    w2b = np.concatenate([W2, b2[None, :]], axis=0).astype(np.float32)
    b1col = np.concatenate([b1, [1.0]]).astype(np.float32).reshape(HB, 1)

    # Banded Toeplitz over two adjacent 128-blocks; avg band folds in the
    # 1/C spatial-mean scale (device computes raw channel sums).
    wa = conv_w[:, 0, 0] / C
    wm = conv_w[:, 1, 0]
    Wb_a = np.zeros((2 * P, 2 * P), np.float32)
    Wb_m = np.zeros((2 * P, 2 * P), np.float32)
    for i in range(2 * P):
        for k in range(7):
            j = i + k - 3
            if 0 <= j < 2 * P:
                Wb_a[i, j] = wa[k]
                Wb_m[i, j] = wm[k]
    cmain = np.concatenate([Wb_a[0:P, 0:P].T, Wb_m[0:P, 0:P].T], axis=1)
    # Corner lhsTs in one [128, 512] tensor. The prev-block ("lo") bands use
    # full K=128 (only rows 125-127 nonzero) so the rhs stays at base
    # partition 0 (PE requires base partition in {0, 32, 64}); the
    # next-block ("hi") bands are K=3 at rows 0-2.
    corn = np.zeros((P, 4 * P), np.float32)
    corn[:, 0:P] = Wb_a[P:2 * P, 0:P].T            # prev-block avg
    corn[:, P:2 * P] = Wb_m[P:2 * P, 0:P].T        # prev-block max
    corn[0:3, 2 * P:3 * P] = Wb_a[0:P, P:2 * P].T[0:3, :]   # next-block avg
    corn[0:3, 3 * P:4 * P] = Wb_m[0:P, P:2 * P].T[0:3, :]   # next-block max
    return {
        "w1sb": w1sb,
        "b1col": np.ascontiguousarray(b1col),
        "w2b": w2b,
        "convmain": np.ascontiguousarray(cmain),
        "convcorner": np.ascontiguousarray(corn),
        "ones": np.ones((P, P), np.float32),
        "ident": np.eye(P, dtype=np.float32),
    }


def kernel(x, W1, b1, W2, b2, conv_w):
    nc = get_nc()
    x = np.asarray(x, np.float32)
    params = _prep_inputs(W1, b1, W2, b2, conv_w)
    in_maps = []
    for c in range(N_CORES):
        m = dict(params)
        m["xb"] = np.ascontiguousarray(x[NB * c:NB * (c + 1)])
        in_maps.append(m)
    _CACHE["last_in_maps"] = in_maps
    res = run_bass_kernel_spmd(nc, in_maps, list(range(N_CORES)))
    _CACHE["last_results"] = res
    return np.concatenate([res.results[c]["out"] for c in range(N_CORES)],
                          axis=0)


def bench(n_iters=30, in_maps=None):
    """Time back-to-back NEFF executions with device-resident inputs.

    Mirrors bass2jax.run_bass_via_pjrt's multi-core path but without buffer
    donation so inputs (incl. zero-filled output buffers) stay reusable
    across iterations; reports amortized per-iteration wall time, which
    bounds true HW exec time from above by the per-dispatch overhead.
    """
    import time
    import jax
    import concourse.mybir as mybir_
    from concourse.bass2jax import (_bass_exec_p, install_neuronx_cc_hook,
                                    partition_id_tensor)
    from jax.experimental.shard_map import shard_map
    from jax.sharding import Mesh, PartitionSpec

    nc = get_nc()
    if in_maps is None:
        in_maps = _CACHE["last_in_maps"]
    install_neuronx_cc_hook()

    partition_name = (nc.partition_id_tensor.name
                      if nc.partition_id_tensor else None)
    in_names, out_names, out_avals, zero_outs = [], [], [], []
    for alloc in nc.m.functions[0].allocations:
        if not isinstance(alloc, mybir_.MemoryLocationSet):
            continue
        name = alloc.memorylocations[0].name
        if alloc.kind == "ExternalInput":
            if name != partition_name:
                in_names.append(name)
        elif alloc.kind == "ExternalOutput":
            shape = tuple(alloc.tensor_shape)
            dtype = mybir_.dt.np(alloc.dtype)
            out_names.append(name)
            out_avals.append(jax.core.ShapedArray(shape, dtype))
            zero_outs.append(np.zeros(shape, dtype))
    n_params = len(in_names)
    all_in_names = list(in_names) + list(out_names)
    if partition_name is not None:
        all_in_names.append(partition_name)

    def _body(*args):
        operands = list(args)
        if partition_name is not None:
            operands.append(partition_id_tensor())
        return tuple(_bass_exec_p.bind(
            *operands,
            out_avals=tuple(out_avals),
            in_names=tuple(all_in_names),
            out_names=tuple(out_names),
            lowering_input_output_aliases=(),
            sim_require_finite=True,
            sim_require_nnan=True,
            nc=nc,
        ))

    devices = jax.devices()[:N_CORES]
    mesh = Mesh(np.asarray(devices), ("core",))
    nin = n_params + len(out_names)
    sharded = jax.jit(shard_map(
        _body, mesh=mesh,
        in_specs=(PartitionSpec("core"),) * nin,
        out_specs=(PartitionSpec("core"),) * len(out_names),
        check_rep=False))

    concat_in = [
        np.concatenate([np.asarray(in_maps[c][nm]) for c in range(N_CORES)],
                       axis=0)
        for nm in in_names
    ]
    concat_zeros = [
        np.zeros((N_CORES * z.shape[0], *z.shape[1:]), z.dtype)
        for z in zero_outs
    ]
    sharding = jax.sharding.NamedSharding(mesh, PartitionSpec("core"))
    dev_args = [jax.device_put(a, sharding) for a in concat_in + concat_zeros]

    out = sharded(*dev_args)
    jax.block_until_ready(out)
    t0 = time.perf_counter()
    for _ in range(n_iters):
        out = sharded(*dev_args)
    jax.block_until_ready(out)
    t1 = time.perf_counter()
    per_iter_ns = (t1 - t0) / n_iters * 1e9
    result = np.asarray(out[0]).reshape(N_CORES * NB, L, C)
    return per_iter_ns, result


# revision 14
# speedup vs baseline: 14.8212x; 14.8212x over previous
"""CBAM kernel for Trainium2, 8-way batch-parallel SPMD.

Computes out = x^2 * (att_c[b,c] + sigmoid(conv(spatial_stats))[b,l]) where
att_c = sigmoid(mlp(mean_L x) + mlp(max_L x)), matching the CBAM reference.

Layout per core: 4 batches; each batch x[4096, 256] lives in SBUF as one
[128, 8192] tensor (partition = l % 128, free column = 256*(l//128) + c).
Engine split per batch:
  PE   : channel-sum (ones-matmul accumulation), transposes, MLP, conv
         (conv over L is a banded-Toeplitz matmul with host-built weights)
  ACT  : spatial sums (copy w/ accum_out), squares, sigmoids, relu
  DVE  : spatial max (one 3D reduce), max-tree folds, final fused
         (att + sig) * x^2 via scalar_tensor_tensor
  POOL : first max-tree fold
"""

import numpy as np
from contextlib import ExitStack

import concourse.bacc as bacc
import concourse.bass as bass
import concourse.tile as tile
import concourse.mybir as mybir
from concourse.bass_utils import run_bass_kernel_spmd

AF = mybir.ActivationFunctionType
ALU = mybir.AluOpType
AX = mybir.AxisListType
FP32 = mybir.dt.float32

N_CORES = 8
B_FULL = 32
NB = B_FULL // N_CORES  # batches per core = 4
L = 4096
C = 256
HID = 16
P = 128
NT = L // P  # 32 L-tiles per batch
SQW = 2048   # ACT square slice width (8 tiles)

_CACHE: dict = {}


def _build_body(ctx: ExitStack, tc, out_d, x_d, w1_d, b1_d, w2b_d, cm_d, cc_d,
                ones_d, id_d, reps=1):
    nc = tc.nc

    const = ctx.enter_context(tc.tile_pool(name="const", bufs=1))
    xpool = ctx.enter_context(tc.tile_pool(name="x", bufs=2))
    mpool = ctx.enter_context(tc.tile_pool(name="maxtree", bufs=2))
    spool = ctx.enter_context(tc.tile_pool(name="stats", bufs=2))
    sqpool = ctx.enter_context(tc.tile_pool(name="sq", bufs=3))
    opool = ctx.enter_context(tc.tile_pool(name="outt", bufs=8))
    dpool = ctx.enter_context(tc.tile_pool(name="dummy", bufs=2))
    apool = ctx.enter_context(tc.tile_pool(name="att", bufs=2))
    pacc = ctx.enter_context(tc.tile_pool(name="pacc", bufs=2, space="PSUM"))
    pwork = ctx.enter_context(tc.tile_pool(name="pwork", bufs=4, space="PSUM"))

    w1 = const.tile([P, 2 * (HID + 1)], FP32)
    nc.sync.dma_start(w1[:], w1_d[:])
    b1 = const.tile([HID + 1, 1], FP32)
    nc.sync.dma_start(b1[:], b1_d[:])
    w2b = const.tile([HID + 1, C], FP32)
    nc.sync.dma_start(w2b[:], w2b_d[:])
    cmain = const.tile([P, 2 * P], FP32)
    nc.sync.dma_start(cmain[:], cm_d[:])
    ccorn = const.tile([P, 4 * P], FP32)
    nc.sync.dma_start(ccorn[:], cc_d[:])
    ones = const.tile([P, P], FP32)
    nc.sync.dma_start(ones[:], ones_d[:])
    ident = const.tile([P, P], FP32)
    nc.sync.dma_start(ident[:], id_d[:])

    for b in [b for _ in range(reps) for b in range(NB)]:
        xb = xpool.tile([P, NT * C], FP32, tag="xb")
        for t in range(NT):
            nc.sync.dma_start(xb[:, C * t:C * (t + 1)],
                              x_d[b, P * t:P * (t + 1), :])

        # ---- channel sum over L: PSUM-accumulated ones-matmuls (PE) ----
        pcs = pacc.tile([1, C], FP32, tag="pcs")
        for t in range(NT):
            nc.tensor.matmul(pcs[:], ones[:, 0:1], xb[:, C * t:C * (t + 1)],
                             start=(t == 0), stop=(t == NT - 1),
                             skip_group_check=True)

        # ---- spatial sum over C: ACT copies with accum_out ----
        sum_s = spool.tile([P, NT], FP32, tag="sum_s")
        for t in range(NT):
            dummy = dpool.tile([P, C], FP32, tag="dummy")
            nc.scalar.activation(dummy[:], xb[:, C * t:C * (t + 1)],
                                 AF.Identity, accum_out=sum_s[:, t:t + 1])

        # ---- spatial max over C: one 3D reduce (DVE) ----
        max_s = spool.tile([P, NT], FP32, tag="max_s")
        nc.vector.tensor_reduce(max_s[:],
                                xb[:].rearrange("p (t c) -> p t c", c=C),
                                axis=AX.X, op=ALU.max)

        # ---- channel max over L: fold tree then transpose+reduce ----
        mb = mpool.tile([P, NT * C // 2], FP32, tag="mb")
        half = NT * C // 2  # 4096
        nc.vector.tensor_max(mb[:], xb[:, 0:half], xb[:, half:2 * half])
        w = half // 2
        while w >= C:
            nc.vector.tensor_max(mb[:, 0:w], mb[:, 0:w], mb[:, w:2 * w])
            w //= 2

        stats_cm = spool.tile([P, 4], FP32, tag="stats_cm")
        for h in range(2):
            pt = pwork.tile([P, P], FP32, tag="pwork")
            nc.tensor.transpose(pt[:], mb[:, P * h:P * (h + 1)], ident[:])
            nc.vector.tensor_reduce(stats_cm[:, 2 * h + 1:2 * h + 2], pt[:],
                                    axis=AX.X, op=ALU.max)

        avg_row = spool.tile([1, C], FP32, tag="avg_row")
        nc.scalar.activation(avg_row[:], pcs[:], AF.Copy, scale=1.0 / L)
        for h in range(2):
            pa = pwork.tile([P, 1], FP32, tag="pwork")
            nc.tensor.transpose(pa[:], avg_row[:, P * h:P * (h + 1)],
                                ident[0:1, 0:1])
            nc.scalar.activation(stats_cm[:, 2 * h:2 * h + 1], pa[:], AF.Copy)

        # ---- shared MLP: att logits broadcast over partitions via matmul ----
        # Row HID (=16) carries a constant: lhsT col 16 is zero, relu bias row
        # 16 is 1.0, so hsb[16, :] = 1, h2[16] = 2 — which multiplies the b2
        # row of w2b to add the 2*b2 term.
        ph = pwork.tile([HID + 1, 2], FP32, tag="pwork")
        nc.tensor.matmul(ph[:], w1[:, 0:HID + 1], stats_cm[:, 0:2],
                         start=True, stop=False, skip_group_check=True)
        nc.tensor.matmul(ph[:], w1[:, HID + 1:2 * (HID + 1)], stats_cm[:, 2:4],
                         start=False, stop=True, skip_group_check=True)
        hsb = spool.tile([HID + 1, 2], FP32, tag="hsb")
        nc.scalar.activation(hsb[:], ph[:], AF.Relu, bias=b1[:])
        h2 = spool.tile([HID + 1, 1], FP32, tag="h2")
        nc.vector.tensor_add(h2[:], hsb[:, 0:1], hsb[:, 1:2])
        h2r = spool.tile([HID + 1, P], FP32, tag="h2r")
        nc.vector.tensor_scalar_mul(h2r[:], ones[0:HID + 1, :], h2[:])
        po = pwork.tile([P, C], FP32, tag="pwork")
        nc.tensor.matmul(po[:], h2r[:], w2b[:], start=True, stop=True,
                         skip_group_check=True)
        att = apool.tile([P, C], FP32, tag="att")
        nc.scalar.activation(att[:], po[:], AF.Sigmoid)

        # ---- spatial conv over L: banded-Toeplitz matmuls ----
        pc = pwork.tile([P, NT], FP32, tag="pwork")
        nc.tensor.matmul(pc[:, :], cmain[:, 0:P], sum_s[:],
                         start=True, stop=False, skip_group_check=True)
        nc.tensor.matmul(pc[:, :], cmain[:, P:2 * P], max_s[:],
                         start=False, stop=False, skip_group_check=True)
        nc.tensor.matmul(pc[:, 1:NT], ccorn[:, 0:P], sum_s[:, 0:NT - 1],
                         start=False, stop=False, skip_group_check=True)
        nc.tensor.matmul(pc[:, 1:NT], ccorn[:, P:2 * P],
                         max_s[:, 0:NT - 1],
                         start=False, stop=False, skip_group_check=True)
        nc.tensor.matmul(pc[:, 0:NT - 1], ccorn[0:3, 2 * P:3 * P],
                         sum_s[0:3, 1:NT],
                         start=False, stop=False, skip_group_check=True)
        nc.tensor.matmul(pc[:, 0:NT - 1], ccorn[0:3, 3 * P:4 * P],
                         max_s[0:3, 1:NT],
                         start=False, stop=True, skip_group_check=True)
        sig = spool.tile([P, NT], FP32, tag="sig")
        nc.scalar.activation(sig[:], pc[:], AF.Sigmoid)

        # ---- final: out = (att + sig) * x^2 ----
        sqs = []
        for s in range(NT * C // SQW):
            sq = sqpool.tile([P, SQW], FP32, tag="sq")
            nc.scalar.activation(sq[:], xb[:, SQW * s:SQW * (s + 1)], AF.Square)
            sqs.append(sq)
        tps = SQW // C  # tiles per square slice
        for t in range(NT):
            ot = opool.tile([P, C], FP32, tag="ot")
            sq = sqs[t // tps]
            off = C * (t % tps)
            nc.vector.scalar_tensor_tensor(ot[:], att[:], sig[:, t:t + 1],
                                           sq[:, off:off + C],
                                           op0=ALU.add, op1=ALU.mult)
            nc.sync.dma_start(out_d[b, P * t:P * (t + 1), :], ot[:])


def _build_nc(reps=1):
    nc = bacc.Bacc("TRN2", target_bir_lowering=False, debug=False,
                   enable_asserts=False, num_devices=N_CORES)
    x_d = nc.dram_tensor("xb", [NB, L, C], FP32, kind="ExternalInput").ap()
    w1_d = nc.dram_tensor("w1sb", [P, 2 * (HID + 1)], FP32, kind="ExternalInput").ap()
    b1_d = nc.dram_tensor("b1col", [HID + 1, 1], FP32, kind="ExternalInput").ap()
    w2b_d = nc.dram_tensor("w2b", [HID + 1, C], FP32, kind="ExternalInput").ap()
    cm_d = nc.dram_tensor("convmain", [P, 2 * P], FP32, kind="ExternalInput").ap()
    cc_d = nc.dram_tensor("convcorner", [P, 4 * P], FP32, kind="ExternalInput").ap()
    ones_d = nc.dram_tensor("ones", [P, P], FP32, kind="ExternalInput").ap()
    id_d = nc.dram_tensor("ident", [P, P], FP32, kind="ExternalInput").ap()
    out_d = nc.dram_tensor("out", [NB, L, C], FP32, kind="ExternalOutput").ap()

    with tile.TileContext(nc) as tc:
        with ExitStack() as ctx:
            _build_body(ctx, tc, out_d, x_d, w1_d, b1_d, w2b_d, cm_d, cc_d,
                        ones_d, id_d, reps=reps)
    nc.compile()
    return nc


def get_nc(reps=1):
    key = f"nc{reps}"
    if key not in _CACHE:
        _CACHE[key] = _build_nc(reps=reps)
    return _CACHE[key]


def _prep_inputs(W1, b1, W2, b2, conv_w):
    """Host-side parameter preprocessing (shared across cores)."""
    W1 = np.asarray(W1, np.float32)
    W2 = np.asarray(W2, np.float32)
    b1 = np.asarray(b1, np.float32)
    b2 = np.asarray(b2, np.float32)
    conv_w = np.asarray(conv_w, np.float32)

    HB = HID + 1
    w1sb = np.zeros((P, 2 * HB), np.float32)
    for h in range(2):
        w1sb[:, HB * h:HB * h + HID] = W1[P * h:P * (h + 1), :]
    w2b = np.concatenate([W2, b2[None, :]], axis=0).astype(np.float32)
    b1col = np.concatenate([b1, [1.0]]).astype(np.float32).reshape(HB, 1)

    # Banded Toeplitz over two adjacent 128-blocks; avg band folds in the
    # 1/C spatial-mean scale (device computes raw channel sums).
    wa = conv_w[:, 0, 0] / C
    wm = conv_w[:, 1, 0]
    Wb_a = np.zeros((2 * P, 2 * P), np.float32)
    Wb_m = np.zeros((2 * P, 2 * P), np.float32)
    for i in range(2 * P):
        for k in range(7):
            j = i + k - 3
            if 0 <= j < 2 * P:
                Wb_a[i, j] = wa[k]
                Wb_m[i, j] = wm[k]
    cmain = np.concatenate([Wb_a[0:P, 0:P].T, Wb_m[0:P, 0:P].T], axis=1)
    # Corner lhsTs in one [128, 512] tensor. The prev-block ("lo") bands use
    # full K=128 (only rows 125-127 nonzero) so the rhs stays at base
    # partition 0 (PE requires base partition in {0, 32, 64}); the
    # next-block ("hi") bands are K=3 at rows 0-2.
    corn = np.zeros((P, 4 * P), np.float32)
    corn[:, 0:P] = Wb_a[P:2 * P, 0:P].T            # prev-block avg
    corn[:, P:2 * P] = Wb_m[P:2 * P, 0:P].T        # prev-block max
    corn[0:3, 2 * P:3 * P] = Wb_a[0:P, P:2 * P].T[0:3, :]   # next-block avg
    corn[0:3, 3 * P:4 * P] = Wb_m[0:P, P:2 * P].T[0:3, :]   # next-block max
    return {
        "w1sb": w1sb,
        "b1col": np.ascontiguousarray(b1col),
        "w2b": w2b,
        "convmain": np.ascontiguousarray(cmain),
        "convcorner": np.ascontiguousarray(corn),
        "ones": np.ones((P, P), np.float32),
        "ident": np.eye(P, dtype=np.float32),
    }


def kernel(x, W1, b1, W2, b2, conv_w):
    nc = get_nc()
    x = np.asarray(x, np.float32)
    params = _prep_inputs(W1, b1, W2, b2, conv_w)
    in_maps = []
    for c in range(N_CORES):
        m = dict(params)
        m["xb"] = np.ascontiguousarray(x[NB * c:NB * (c + 1)])
        in_maps.append(m)
    _CACHE["last_in_maps"] = in_maps
    res = run_bass_kernel_spmd(nc, in_maps, list(range(N_CORES)))
    _CACHE["last_results"] = res
    return np.concatenate([res.results[c]["out"] for c in range(N_CORES)],
                          axis=0)


def _pjrt_exec(nc, in_maps, n_warm=2, n_time=8):
    """Build a sharded jit for nc, run it, return (best_wall_s, result)."""
    import time
    import jax
    import concourse.mybir as mybir_
    from concourse.bass2jax import (_bass_exec_p, install_neuronx_cc_hook,
                                    partition_id_tensor)
    from jax.experimental.shard_map import shard_map
    from jax.sharding import Mesh, PartitionSpec

    install_neuronx_cc_hook()
    partition_name = (nc.partition_id_tensor.name
                      if nc.partition_id_tensor else None)
    in_names, out_names, out_avals = [], [], []
    for alloc in nc.m.functions[0].allocations:
        if not isinstance(alloc, mybir_.MemoryLocationSet):
            continue
        name = alloc.memorylocations[0].name
        if alloc.kind == "ExternalInput":
            if name != partition_name:
                in_names.append(name)
        elif alloc.kind == "ExternalOutput":
            out_names.append(name)
            out_avals.append(jax.core.ShapedArray(
                tuple(alloc.tensor_shape), mybir_.dt.np(alloc.dtype)))
    n_params = len(in_names)
    all_in_names = list(in_names) + list(out_names)
    if partition_name is not None:
        all_in_names.append(partition_name)

    def _body(*args):
        operands = list(args)
        if partition_name is not None:
            operands.append(partition_id_tensor())
        return tuple(_bass_exec_p.bind(
            *operands,
            out_avals=tuple(out_avals),
            in_names=tuple(all_in_names),
            out_names=tuple(out_names),
            lowering_input_output_aliases=(),
            sim_require_finite=True,
            sim_require_nnan=True,
            nc=nc,
        ))

    devices = jax.devices()[:N_CORES]
    mesh = Mesh(np.asarray(devices), ("core",))
    nin = n_params + len(out_names)
    sharding = jax.sharding.NamedSharding(mesh, PartitionSpec("core"))
    fn = jax.jit(shard_map(
        _body, mesh=mesh,
        in_specs=(PartitionSpec("core"),) * nin,
        out_specs=(PartitionSpec("core"),) * len(out_names),
        check_rep=False))
    dev_args = [
        jax.device_put(np.concatenate(
            [np.asarray(in_maps[c][nm]) for c in range(N_CORES)], axis=0),
            sharding)
        for nm in in_names
    ]
    for av in out_avals:
        z = np.zeros((N_CORES * av.shape[0], *av.shape[1:]), av.dtype)
        dev_args.append(jax.device_put(z, sharding))

    for _ in range(n_warm):
        out = fn(*dev_args)
        jax.block_until_ready(out)
    best = float("inf")
    for _ in range(n_time):
        t0 = time.perf_counter()
        out = fn(*dev_args)
        jax.block_until_ready(out)
        best = min(best, time.perf_counter() - t0)
    result = np.asarray(out[0]).reshape(N_CORES * NB, L, C)
    return best, result


def bench_repeat(reps=8, n_time=10, in_maps=None):
    """Isolate device exec time: time a module doing the work `reps` times
    in-kernel vs once; slope = steady-state HW time per execution."""
    if in_maps is None:
        in_maps = _CACHE["last_in_maps"]
    t1, _ = _pjrt_exec(get_nc(1), in_maps, n_time=n_time)
    tr, result = _pjrt_exec(get_nc(reps), in_maps, n_time=n_time)
    per_exec_ns = (tr - t1) / (reps - 1) * 1e9
    return per_exec_ns, result, t1 * 1e9, tr * 1e9


def bench(n_iters=30, in_maps=None):
    """Time back-to-back NEFF executions with device-resident inputs.

    Mirrors bass2jax.run_bass_via_pjrt's multi-core path but without buffer
    donation so inputs (incl. zero-filled output buffers) stay reusable
    across iterations; reports amortized per-iteration wall time, which
    bounds true HW exec time from above by the per-dispatch overhead.
    """
    import time
    import jax
    import concourse.mybir as mybir_
    from concourse.bass2jax import (_bass_exec_p, install_neuronx_cc_hook,
                                    partition_id_tensor)
    from jax.experimental.shard_map import shard_map
    from jax.sharding import Mesh, PartitionSpec

    nc = get_nc()
    if in_maps is None:
        in_maps = _CACHE["last_in_maps"]
    install_neuronx_cc_hook()

    partition_name = (nc.partition_id_tensor.name
                      if nc.partition_id_tensor else None)
    in_names, out_names, out_avals, zero_outs = [], [], [], []
    for alloc in nc.m.functions[0].allocations:
        if not isinstance(alloc, mybir_.MemoryLocationSet):
            continue
        name = alloc.memorylocations[0].name
        if alloc.kind == "ExternalInput":
            if name != partition_name:
                in_names.append(name)
        elif alloc.kind == "ExternalOutput":
            shape = tuple(alloc.tensor_shape)
            dtype = mybir_.dt.np(alloc.dtype)
            out_names.append(name)
            out_avals.append(jax.core.ShapedArray(shape, dtype))
            zero_outs.append(np.zeros(shape, dtype))
    n_params = len(in_names)
    all_in_names = list(in_names) + list(out_names)
    if partition_name is not None:
        all_in_names.append(partition_name)

    def _body(*args):
        operands = list(args)
        if partition_name is not None:
            operands.append(partition_id_tensor())
        return tuple(_bass_exec_p.bind(
            *operands,
            out_avals=tuple(out_avals),
            in_names=tuple(all_in_names),
            out_names=tuple(out_names),
            lowering_input_output_aliases=(),
            sim_require_finite=True,
            sim_require_nnan=True,
            nc=nc,
        ))

    devices = jax.devices()[:N_CORES]
    mesh = Mesh(np.asarray(devices), ("core",))
    nin = n_params + len(out_names)
    sharded = jax.jit(shard_map(
        _body, mesh=mesh,
        in_specs=(PartitionSpec("core"),) * nin,
        out_specs=(PartitionSpec("core"),) * len(out_names),
        check_rep=False))

    concat_in = [
        np.concatenate([np.asarray(in_maps[c][nm]) for c in range(N_CORES)],
                       axis=0)
        for nm in in_names
    ]
    concat_zeros = [
        np.zeros((N_CORES * z.shape[0], *z.shape[1:]), z.dtype)
        for z in zero_outs
    ]
    sharding = jax.sharding.NamedSharding(mesh, PartitionSpec("core"))
    dev_args = [jax.device_put(a, sharding) for a in concat_in + concat_zeros]

    out = sharded(*dev_args)
    jax.block_until_ready(out)
    t0 = time.perf_counter()
    for _ in range(n_iters):
        out = sharded(*dev_args)
    jax.block_until_ready(out)
    t1 = time.perf_counter()
    per_iter_ns = (t1 - t0) / n_iters * 1e9
    result = np.asarray(out[0]).reshape(N_CORES * NB, L, C)
    return per_iter_ns, result
